# revision 63
# baseline (speedup 1.0000x reference)
"""Trainium2 Bass kernel for a single transformer encoder layer.

Problem: src [8, 1024, 512], 8-head self-attention (d=512, hd=64),
FFN 512->128->512, two post-residual LayerNorms, eval mode.

Sharding: data-parallel over batch -- each of the 8 NeuronCores gets one
batch element [1024, 512] and runs the full layer on it.

Optimized for the TimelineSim cost model:
  - fp8e4m3 + DoubleRow matmuls for QKV / attn@V / out-proj (0.5 cyc/row)
  - f32r matmuls for scores / FFN (1 cyc/row at N=512)
  - softmax exp split across ACT (native exp -> fp8) and DVE (one-op
    Schraudolph: int8 = round(s*log2e + B), bitcast fp8e4m3)
  - PSUM-touching vector work lives on ACT/DVE (Pool has no PSUM port
    and no AP-scalar ops); Pool carries SBUF-side broadcast/scale work
  - per-head softmax denominator: ones-column in the fp8 V operand; the
    reciprocal row is partition-broadcast and folded into the required
    PSUM->fp8 convert
  - host folds: out_proj bias + v-bias@Wo into src residual; LN1 gamma
    into w1; LN1 beta into FFN2 bias; v scaled 16x so ctx8 avoids fp8
    subnormals (1/16 folded into woT8)
"""

import sys

for _p in ("/opt/trn_rl_repo",):
    if _p not in sys.path:
        sys.path.insert(0, _p)

import numpy as np
import ml_dtypes

import concourse.bass as bass
import concourse.mybir as mybir
import concourse.tile as tile
from concourse import bacc
from concourse.bass_utils import run_bass_kernel_spmd
from concourse.masks import make_identity

F32 = mybir.dt.float32
F32R = mybir.dt.float32r
F8 = mybir.dt.float8e4
BF16 = mybir.dt.bfloat16
I8 = mybir.dt.int8
ALU = mybir.AluOpType
ACTF = mybir.ActivationFunctionType
DR = mybir.MatmulPerfMode.DoubleRow

B = 8          # batch == number of cores
S = 1024       # sequence length
D = 512        # model dim
H = 8          # heads
HD = 64        # head dim
FF = 128       # ffn dim
EPS = 1e-5
P = 128        # partitions
SC = S // P    # 8 s-chunks
DC = D // P    # 4 d-chunks
SB = S // 512  # 2 s-blocks of 512
VS = 16.0      # v scale (fp8 subnormal avoidance), 1/VS folded into woT8

# Schraudolph exp -> fp8e4m3 bits: int8 = round(s * log2e + B8)
SCH_A = 1.4426950408889634
SCH_B = 55.54

# exp tiles (h, sk) handled by DVE instead of ACT (tuning knob)
EXP_DVE = frozenset({i for i in range(16, 64) if i % 8 in (1, 5)} | {20, 36, 52})

_CACHED = {}


def dve_rsqrt(nc, out_ap, var_ap, tmp_pool, n, eng, n_iter=1):
    """out = 1/sqrt(var + EPS) via bit-trick seed + Newton steps."""
    ti = tmp_pool.tile([P, n], mybir.dt.int32, tag="rsq_i", name=f"rsq_i{n}")
    tv = tmp_pool.tile([P, n], F32, tag="rsq_v", name=f"rsq_v{n}")
    ty = tmp_pool.tile([P, n], F32, tag="rsq_y", name=f"rsq_y{n}")
    tt = tmp_pool.tile([P, n], F32, tag="rsq_t", name=f"rsq_t{n}")
    eng.tensor_scalar_add(tv[:], var_ap, EPS)
    eng.tensor_scalar(
        out=ti[:], in0=tv[:].bitcast(mybir.dt.int32), scalar1=1, scalar2=None,
        op0=ALU.logical_shift_right,
    )
    eng.tensor_scalar(
        out=ti[:], in0=ti[:], scalar1=0x5F3759DF, scalar2=-1,
        op0=ALU.subtract, op1=ALU.mult,
    )
    eng.tensor_copy(out=ty[:], in_=ti[:].bitcast(F32))
    for _ in range(n_iter):
        eng.tensor_tensor(out=tt[:], in0=ty[:], in1=ty[:], op=ALU.mult)
        eng.tensor_tensor(out=tt[:], in0=tt[:], in1=tv[:], op=ALU.mult)
        eng.tensor_scalar(
            out=tt[:], in0=tt[:], scalar1=-0.5, scalar2=1.5,
            op0=ALU.mult, op1=ALU.add,
        )
        eng.tensor_tensor(out=ty[:], in0=ty[:], in1=tt[:], op=ALU.mult)
    eng.tensor_copy(out=out_ap, in_=ty[:])


def dve_rsqrt2(nc, out_ap, var_ap, tmp_pool, n, eng):
    dve_rsqrt(nc, out_ap, var_ap, tmp_pool, n, eng, n_iter=1)


def build_bass():
    nc = bacc.Bacc(None, target_bir_lowering=False)

    # ---- DRAM I/O ----------------------------------------------------
    a_srcT8 = nc.declare_dram_parameter("srcT8", [P, 2, 2, S], F8, False)
    a_winT8 = nc.declare_dram_parameter("winT8", [P, 2, 2, 3 * D], F8, False)
    a_woT8 = nc.declare_dram_parameter("woT8", [P, 2, 2, D], F8, False)
    a_srcpp = nc.declare_dram_parameter("srcpp", [S, D], F32R, False)
    a_w1T = nc.declare_dram_parameter("w1T", [P, DC, FF], BF16, False)
    a_w2T = nc.declare_dram_parameter("w2T", [FF, D], F32R, False)
    a_inbqk = nc.declare_dram_parameter("inbqk", [2 * D], F32R, False)
    a_b1p = nc.declare_dram_parameter("b1p", [FF], F32, False)
    a_b2r = nc.declare_dram_parameter("b2r", [D], F32R, False)
    a_g1 = nc.declare_dram_parameter("g1", [D], F32, False)
    a_g2 = nc.declare_dram_parameter("g2", [D], F32, False)
    a_be2 = nc.declare_dram_parameter("be2", [D], F32, False)
    a_ones = nc.declare_dram_parameter("ones", [D], F32R, False)
    a_ident = nc.declare_dram_parameter("ident", [P, P], F32R, False)
    a_out = nc.declare_dram_parameter("out", [S, D], F32, True)

    def bcast(vec, n):
        vec_ap = vec[:]
        return bass.AP(
            tensor=vec_ap.tensor, offset=vec_ap.offset, ap=[[0, P], [1, n]]
        )

    with tile.TileContext(nc) as tc:
        with (
            tc.tile_pool(name="persist", bufs=1) as persist,
            tc.tile_pool(name="small", bufs=1) as small,
        ):
            # ---- persistent tiles -----------------------------------
            t_srcT8 = persist.tile([P, 2, 2, S], F8, tag="srcT8")
            t_winT8 = persist.tile([P, 2, 2, 3 * D], F8, tag="winT8")
            t_woT8 = persist.tile([P, 2, 2, D], F8, tag="woT8")
            t_srcpp = persist.tile([P, SC, D], F32R, tag="srcpp")
            t_qkT = [persist.tile([P, S], F32R, tag=f"qkT{c}", name=f"qkT{c}")
                     for c in range(8)]
            # vaug8[i]: [p, j(2), h(8), 80]; col 64 = ones (den), 65.. pad
            t_vaug8 = [persist.tile([P, 2, H, 80], F8, tag=f"vaug{i}",
                                    name=f"vaug{i}") for i in range(4)]
            # ctx8[t]: c-chunk pair t: [p, j(2), sb(2), 512]
            t_ctx8 = [persist.tile([P, 2, SB, 512], F8, tag=f"ctx8{t}",
                                   name=f"ctx8{t}") for t in range(2)]
            t_w1T = persist.tile([P, DC, FF], BF16, tag="w1T")
            t_w2T = persist.tile([FF, D], F32R, tag="w2T")
            t_g1b = persist.tile([P, D], F32, tag="g1b")
            t_g2b = persist.tile([P, D], F32, tag="g2b")
            t_be2b = persist.tile([P, D], F32, tag="be2b")

            t_inbP = small.tile([P, 8], F32, tag="inbP")  # qk bias, chunk cols
            t_ones = small.tile([1, D], F32R, tag="ones")
            t_b1p = small.tile([FF, 1], F32, tag="b1p")
            t_b2r = small.tile([1, D], F32R, tag="b2r")
            t_ident = small.tile([P, P], F32R, tag="ident")

            # LN stats scratch
            t_bn1 = small.tile([P, SC, 6], F32, tag="bn1")
            t_mv1 = small.tile([P, SC, 2], F32, tag="mv1")
            t_rsig1 = small.tile([P, SC], F32, tag="rsig1")
            t_bp1 = small.tile([P, SC], F32, tag="bp1")
            t_eps = small.tile([P, 1], F32, tag="eps")
            t_bn2 = small.tile([P, SC, 6], F32, tag="bn2")
            t_mv2 = small.tile([P, SC, 2], F32, tag="mv2")
            t_rsig2 = small.tile([P, SC], F32, tag="rsig2")
            t_nmu2 = small.tile([P, SC], F32, tag="nmu2")
            t_nr2 = small.tile([P, SC], F32, tag="nr2")

            # ---- load DMAs (SP queue) -------------------------------
            nc.sync.dma_start(out=t_winT8[:, 0, :, :], in_=a_winT8[:, 0, :, :])
            nc.sync.dma_start(out=t_srcT8[:, 0, :, :], in_=a_srcT8[:, 0, :, :])
            nc.sync.dma_start(out=t_winT8[:, 1, :, :], in_=a_winT8[:, 1, :, :])
            nc.sync.dma_start(out=t_srcT8[:, 1, :, :], in_=a_srcT8[:, 1, :, :])
            nc.sync.dma_start(
                out=t_inbP[:],
                in_=a_inbqk[:].bitcast(F32).rearrange("(c p) -> p c", p=P),
            )
            nc.sync.dma_start(out=t_ones[:], in_=a_ones[None, :])
            nc.sync.dma_start(out=t_woT8[:], in_=a_woT8[:, :, :, :])
            nc.sync.dma_start(
                out=t_srcpp[:], in_=a_srcpp[:, :].rearrange("(c p) d -> p c d", p=P)
            )
            nc.sync.dma_start(out=t_w1T[:], in_=a_w1T[:, :, :])
            nc.sync.dma_start(out=t_w2T[:], in_=a_w2T[:, :])
            nc.sync.dma_start(out=t_g1b[:], in_=bcast(a_g1, D))
            nc.sync.dma_start(out=t_g2b[:], in_=bcast(a_g2, D))
            nc.sync.dma_start(out=t_be2b[:], in_=bcast(a_be2, D))
            nc.sync.dma_start(out=t_b1p[:], in_=a_b1p[:, None])
            nc.sync.dma_start(out=t_b2r[:], in_=a_b2r[None, :])
            nc.vector.memset(t_eps[:], EPS)
            nc.sync.dma_start(out=t_ident[:], in_=a_ident[:, :])
            # ones columns of vaug8 (fp8 1.0)
            for i in range(4):
                nc.gpsimd.memset(t_vaug8[i][:, :, :, 64:65].bitcast(I8), 0x38)

            # ---- phases 1+2: QKV (fp8 DR) interleaved with attention --
            with (
                tc.tile_pool(name="ps1", bufs=2, space="PSUM") as ps1,
                tc.tile_pool(name="pssc", bufs=2, space="PSUM") as pssc,
                tc.tile_pool(name="psctx", bufs=1, space="PSUM") as psctx,
                tc.tile_pool(name="expb", bufs=2) as expb,
                tc.tile_pool(name="rbb", bufs=2) as rbb,
                tc.tile_pool(name="rdn", bufs=2) as rdn,
            ):
                def emit_qk(cc, eng):
                    for sb in range(SB):
                        ps = ps1.tile([P, 512], F32, tag="mm", name=f"qk{cc}_{sb}")
                        for g in range(2):
                            nc.tensor.matmul(
                                ps[:],
                                lhsT=t_winT8[:, g, :, cc * P:(cc + 1) * P],
                                rhs=t_srcT8[:, g, :, sb * 512:(sb + 1) * 512],
                                start=(g == 0), stop=(g == 1), perf_mode=DR,
                            )
                        if eng == "act":
                            nc.scalar.activation(
                                out=t_qkT[cc][:, sb * 512:(sb + 1) * 512],
                                in_=ps[:], func=ACTF.Identity,
                                bias=t_inbP[:, cc:cc + 1],
                            )
                        else:
                            nc.vector.tensor_scalar_add(
                                t_qkT[cc][:, sb * 512:(sb + 1) * 512],
                                ps[:], t_inbP[:, cc:cc + 1],
                            )

                def emit_v(sc, eng):
                    ps = ps1.tile([P, 512], F32, tag="mm", name=f"v{sc}")
                    for g in range(2):
                        nc.tensor.matmul(
                            ps[:],
                            lhsT=t_srcT8[:, g, :, sc * P:(sc + 1) * P],
                            rhs=t_winT8[:, g, :, 2 * D:3 * D],
                            start=(g == 0), stop=(g == 1), perf_mode=DR,
                        )
                    if eng == "act":
                        nc.scalar.activation(
                            out=t_vaug8[sc // 2][:, sc % 2, :, 0:HD],
                            in_=ps[:].rearrange("p (h d) -> p h d", h=H),
                            func=ACTF.Identity, scale=VS,
                        )
                    else:
                        nc.vector.tensor_scalar_mul(
                            t_vaug8[sc // 2][:, sc % 2, :, 0:HD],
                            ps[:].rearrange("p (h d) -> p h d", h=H), VS,
                        )

                def emit_norm(h, cps):
                    # rden = 1/den ; rb = broadcast ; ctx8 = ctx * rb (fp8)
                    # split per s-block so the three-engine chain pipelines
                    t = h // 4
                    j = (h // 2) % 2
                    p0 = (h % 2) * HD
                    for sb in range(SB):
                        rden = rdn.tile([1, 512], F32, tag=f"rden{sb}",
                                        name=f"rd{h}_{sb}")
                        nc.vector.reciprocal(out=rden[:],
                                             in_=cps[HD:HD + 1, sb, :])
                        rb = rbb.tile([HD, 512], F32, tag=f"rb{sb}",
                                      name=f"rb{h}_{sb}")
                        nc.gpsimd.partition_broadcast(rb[:], rden[:])
                        nc.vector.tensor_tensor(
                            out=t_ctx8[t][p0:p0 + HD, j, sb, :],
                            in0=cps[0:HD, sb, :], in1=rb[:], op=ALU.mult,
                        )

                # head-0 chunks + first v pairs before the head loop;
                # the rest interleaves with head processing below
                emit_qk(0, "act")
                emit_qk(4, "act")
                for sc in range(4):
                    emit_v(sc, "act" if sc < 2 else "dve")

                # QKV work injected at (head, sk) slots:
                inject = {
                    (0, 1): lambda: emit_v(4, "dve"),
                    (0, 3): lambda: emit_v(5, "dve"),
                    (0, 5): lambda: (emit_v(6, "dve"), emit_v(7, "dve")),
                    (1, 1): lambda: emit_qk(1, "dve"),
                    (1, 5): lambda: emit_qk(5, "dve"),
                    (2, 1): lambda: emit_qk(2, "dve"),
                    (2, 5): lambda: emit_qk(6, "dve"),
                    (3, 1): lambda: emit_qk(3, "dve"),
                    (3, 5): lambda: emit_qk(7, "dve"),
                }

                pend = None  # prev head awaiting attnV: (h, exp tiles)
                pcps = None  # prev head ctx psum awaiting normalize
                for h in range(H):
                    qc = h // 2
                    kc = 4 + h // 2
                    po = (h % 2) * HD
                    exps = [expb.tile([P, 2, SB, 512], F8, tag=f"e{i}",
                                      name=f"e_{h}_{i}") for i in range(4)]
                    for sk in range(SC):
                        sps = pssc.tile([P, S], F32, tag="sc",
                                        name=f"sc_{h}_{sk}")
                        for sb in range(SB):
                            nc.tensor.matmul(
                                sps[:, sb * 512:(sb + 1) * 512],
                                lhsT=t_qkT[kc][po:po + HD, sk * P:(sk + 1) * P],
                                rhs=t_qkT[qc][po:po + HD, sb * 512:(sb + 1) * 512],
                                start=True, stop=True,
                            )
                        slot = exps[sk // 2][:, sk % 2, :, :]
                        if h * 8 + sk in EXP_DVE:
                            nc.vector.tensor_scalar(
                                out=slot.bitcast(I8), in0=sps[:],
                                scalar1=SCH_A * 0.125, scalar2=SCH_B,
                                op0=ALU.mult, op1=ALU.add,
                            )
                        else:
                            nc.scalar.activation(
                                out=slot, in_=sps[:], func=ACTF.Exp,
                                bias=0.0, scale=0.125,
                            )
                        if (h, sk) in inject:
                            inject[(h, sk)]()
                        if pend is not None and 3 <= sk <= 6:
                            # spread prev head attnV chain MMs into the
                            # PE idle slots between our scores MMs
                            i = sk - 3
                            ph, pexps = pend
                            if i == 0:
                                pcps = psctx.tile([HD + 1, SB, 512], F32,
                                                  tag="ctx", name=f"ctx_{ph}")
                            for sb in range(SB):
                                nc.tensor.matmul(
                                    pcps[:, sb, :],
                                    lhsT=t_vaug8[i][:, :, ph, 0:HD + 1],
                                    rhs=pexps[i][:, :, sb, :],
                                    start=(i == 0), stop=(i == 3),
                                    perf_mode=DR, skip_group_check=True,
                                )
                            if i == 3:
                                pend = None
                    if pcps is not None:
                        emit_norm(h - 1, pcps)
                        pcps = None
                    pend = (h, exps)
                # final head: attnV + normalize
                ph, pexps = pend
                cps = psctx.tile([HD + 1, SB, 512], F32, tag="ctx",
                                 name=f"ctx_{ph}")
                for i in range(4):
                    for sb in range(SB):
                        nc.tensor.matmul(
                            cps[:, sb, :],
                            lhsT=t_vaug8[i][:, :, ph, 0:HD + 1],
                            rhs=pexps[i][:, :, sb, :],
                            start=(i == 0), stop=(i == 3),
                            perf_mode=DR, skip_group_check=True,
                        )
                emit_norm(ph, cps)

            # ---- phases 3-5: out-proj, LN1, FFN, LN2, store ---------
            with (
                tc.tile_pool(name="pso", bufs=3, space="PSUM") as pso,
                tc.tile_pool(name="psh1", bufs=1, space="PSUM") as psh1,
                tc.tile_pool(name="psf2", bufs=4, space="PSUM") as psf2,
                tc.tile_pool(name="post", bufs=1) as post,
                tc.tile_pool(name="scr", bufs=2) as scr,
                tc.tile_pool(name="rsq", bufs=2) as rsq,
            ):
                t_x = post.tile([P, SC, D], F32, tag="x")
                t_xhat = post.tile([P, SC, D], BF16, tag="xhat")
                t_xg = post.tile([P, SC, D], F32R, tag="xg")
                t_xT = post.tile([P, SC, DC, P], BF16, tag="xT")
                t_xT2 = post.tile([P, SC, DC, P], BF16, tag="xT2")
                t_dscr = post.tile([P, SC], BF16, tag="dscr")
                t_h1T = post.tile([FF, S], F32R, tag="h1T")

                # out-proj + residual(identity-MM) + LN1 stats, per q-chunk
                for qc in range(SC):
                    sb = qc // 4
                    off = (qc % 4) * P
                    ps = pso.tile([P, D], F32, tag="op", name=f"op{qc}")
                    for t in range(2):
                        nc.tensor.matmul(
                            ps[:],
                            lhsT=t_ctx8[t][:, :, sb, off:off + P],
                            rhs=t_woT8[:, t, :, :],
                            start=(t == 0), stop=False, perf_mode=DR,
                        )
                    nc.tensor.matmul(
                        ps[:], lhsT=t_ident[:],
                        rhs=t_srcpp[:, qc, :],
                        start=False, stop=True,
                    )
                    nc.scalar.activation(
                        out=t_x[:, qc, :], in_=ps[:], func=ACTF.Identity,
                    )
                    nc.vector.bn_stats(out=t_bn1[:, qc, :], in_=t_x[:, qc, :])
                    nc.vector.bn_aggr(out=t_mv1[:, qc, :], in_=t_bn1[:, qc, :])

                dve_rsqrt(nc, t_rsig1[:], t_mv1[:, :, 1], rsq, SC,
                          eng=nc.vector, n_iter=2)
                nc.vector.scalar_tensor_tensor(
                    out=t_bp1[:], in0=t_mv1[:, :, 0], scalar=-1.0,
                    in1=t_rsig1[:], op0=ALU.mult, op1=ALU.mult,
                )
                for qc in range(SC):
                    # xhat = x*rsig + bp  (LN1 apply, ACT scale/bias, psum in)
                    nc.scalar.activation(
                        out=t_xhat[:, qc, :], in_=t_x[:, qc, :],
                        func=ACTF.Identity,
                        bias=t_bp1[:, qc:qc + 1],
                        scale=t_rsig1[:, qc:qc + 1],
                    )
                    nc.gpsimd.tensor_tensor(
                        out=t_xg[:, qc, :], in0=t_xhat[:, qc, :],
                        in1=t_g1b[:], op=ALU.mult,
                    )
                for half in range(2):
                    for qc in range(half * 4, half * 4 + 4):
                        # XBAR transpose sandwiched between TRACKED DMAs on
                        # the same FIFO queue: dma_start_transpose's own
                        # deps are not tracked by the tile framework, so a
                        # tracked 1-col read (waits the xhat producer)
                        # orders its start, and a tracked full copy (which
                        # FFN1 reads) orders its completion.
                        nc.scalar.dma_start(
                            out=t_dscr[:, qc:qc + 1],
                            in_=t_xhat[:, qc, 0:1],
                        )
                        nc.scalar.dma_start_transpose(
                            out=t_xT[:, qc, :, :],
                            in_=t_xhat[:, qc, :],
                        )
                    # FFN1 for this half's s-block
                    ps_h = psh1.tile([FF, 512], F32, tag="h1", name=f"h1_{half}")
                    for qx in range(4):
                        qc = half * 4 + qx
                        for dc in range(DC):
                            nc.tensor.matmul(
                                ps_h[:, qx * P:(qx + 1) * P],
                                lhsT=t_w1T[:, dc, :],
                                rhs=t_xT[:, qc, dc, :],
                                start=(dc == 0), stop=(dc == DC - 1),
                            )
                    nc.scalar.activation(
                        out=t_h1T[:, half * 512:(half + 1) * 512], in_=ps_h[:],
                        func=ACTF.Relu, bias=t_b1p[:], scale=1.0,
                    )
                    # FFN2 + residual(identity-MM) + LN2, in chunk-pairs
                    for pair in range(2):
                        q0 = half * 4 + pair * 2
                        pslc = slice(q0, q0 + 2)
                        ps_f2 = {}
                        for qc in (q0, q0 + 1):
                            ps2 = psf2.tile([P, D], F32, tag="f2",
                                            name=f"f2{qc}")
                            nc.tensor.matmul(
                                ps2[:],
                                lhsT=t_h1T[:, qc * P:(qc + 1) * P],
                                rhs=t_w2T[:],
                                start=True, stop=False,
                            )
                            nc.tensor.matmul(
                                ps2[:], lhsT=t_ones[:, 0:P], rhs=t_b2r[:],
                                start=False, stop=False,
                            )
                            nc.tensor.matmul(
                                ps2[:], lhsT=t_ident[:],
                                rhs=t_xg[:, qc, :],
                                start=False, stop=True,
                            )
                            nc.vector.bn_stats(out=t_bn2[:, qc, :], in_=ps2[:])
                            nc.vector.bn_aggr(out=t_mv2[:, qc, :],
                                              in_=t_bn2[:, qc, :])
                            ps_f2[qc] = ps2
                        dve_rsqrt(nc, t_rsig2[:, pslc], t_mv2[:, pslc, 1],
                                  rsq, 2, eng=nc.vector)
                        nc.vector.tensor_scalar_mul(
                            t_nmu2[:, pslc], t_mv2[:, pslc, 0], -1.0
                        )
                        nc.vector.tensor_tensor(
                            out=t_nr2[:, pslc], in0=t_nmu2[:, pslc],
                            in1=t_rsig2[:, pslc], op=ALU.mult,
                        )
                        for qc in (q0, q0 + 1):
                            tz = scr.tile([P, D], F32, tag="tz", name=f"tz{qc}")
                            tg = scr.tile([P, D], F32, tag="tg", name=f"tg{qc}")
                            to = scr.tile([P, D], F32, tag="to", name=f"to{qc}")
                            # tz = x2*rsig2 - mu2*rsig2 ; out = tz*g2 + be2
                            nc.scalar.activation(
                                out=tz[:], in_=ps_f2[qc][:],
                                func=ACTF.Identity,
                                bias=t_nr2[:, qc:qc + 1],
                                scale=t_rsig2[:, qc:qc + 1],
                            )
                            eng_a = nc.gpsimd if qc % 2 == 0 else nc.vector
                            eng_b = nc.vector if qc % 2 == 0 else nc.gpsimd
                            eng_a.tensor_tensor(
                                out=tg[:], in0=tz[:], in1=t_g2b[:], op=ALU.mult,
                            )
                            eng_b.tensor_tensor(
                                out=to[:], in0=tg[:], in1=t_be2b[:], op=ALU.add,
                            )
                            nc.sync.dma_start(
                                out=a_out[qc * P:(qc + 1) * P, :], in_=to[:],
                            )

    nc.finalize()
    return nc


def _prep_in_maps(inputs):
    F8NP = ml_dtypes.float8_e4m3fn
    src = np.ascontiguousarray(np.asarray(inputs["src"], dtype=np.float32))
    win = np.asarray(inputs["in_proj_w"], dtype=np.float32)
    inb = np.asarray(inputs["in_proj_b"], dtype=np.float32)
    wo = np.asarray(inputs["out_proj_w"], dtype=np.float32)
    outb = np.asarray(inputs["out_proj_b"], dtype=np.float32)
    w1 = np.asarray(inputs["w1"], dtype=np.float32)
    b1 = np.asarray(inputs["b1"], dtype=np.float32)
    w2 = np.asarray(inputs["w2"], dtype=np.float32)
    b2 = np.asarray(inputs["b2"], dtype=np.float32)
    g1 = np.asarray(inputs["g1"], dtype=np.float32)
    be1 = np.asarray(inputs["be1"], dtype=np.float32)
    g2 = np.asarray(inputs["g2"], dtype=np.float32)
    be2 = np.asarray(inputs["be2"], dtype=np.float32)

    def dinter(a):
        # [d, m] -> [p, g, j, m] with d = g*256 + j*128 + p
        d, m = a.shape
        return np.ascontiguousarray(
            a.reshape(2, 2, P, m).transpose(2, 0, 1, 3)
        )

    winT8 = dinter(win.T).astype(F8NP)              # [128, 2, 2, 1536]
    woT8 = dinter(wo.T / VS).astype(F8NP)           # [128, 2, 2, 512]
    w1T = np.ascontiguousarray(
        (w1 * g1[None, :]).T.reshape(DC, P, FF).transpose(1, 0, 2)
    ).astype(ml_dtypes.bfloat16)                    # [128, 4, 128] bf16
    w2T = np.ascontiguousarray(w2.T)                # [128, 512]
    b1p = (b1 + w1 @ be1).astype(np.float32)
    b2r = (b2 + be1).astype(np.float32)
    # residual fold: src + out_proj_b + v_bias @ Wo^T
    resfold = (outb + inb[2 * D:] @ wo.T).astype(np.float32)

    shared = dict(
        winT8=winT8, woT8=woT8, w1T=w1T, w2T=w2T,
        inbqk=inb[:2 * D].copy(), b1p=b1p, b2r=b2r,
        g1=g1, g2=g2, be2=be2, ones=np.ones((D,), np.float32),
        ident=np.eye(P, dtype=np.float32),
    )
    in_maps = []
    for i in range(B):
        m = dict(shared)
        m["srcT8"] = dinter(np.ascontiguousarray(src[i].T)).astype(F8NP)
        m["srcpp"] = (src[i] + resfold[None, :]).astype(np.float32)
        in_maps.append(m)
    return in_maps


def _run(inputs, trace=False):
    if "nc" not in _CACHED:
        _CACHED["nc"] = build_bass()
    nc = _CACHED["nc"]
    in_maps = _prep_in_maps(inputs)
    res = run_bass_kernel_spmd(nc, in_maps, list(range(B)), trace=trace)
    out = np.stack([np.asarray(res.results[i]["out"]) for i in range(B)])
    return out.astype(np.float32), res


def kernel(**inputs):
    out, _ = _run(inputs, trace=False)
    return out
